# revision 1
# baseline (speedup 1.0000x reference)
"""Trainium2 Bass kernel for nn_NeuralALU (batched byte-encoded 32-bit add).

The reference network computes, per batch element, a chain of table-lookup
matmuls + sharp softmaxes (scale=100) over exactly-one-hot byte encodings.
Because the inputs are exact one-hots, the float pipeline collapses to a
discrete algorithm (validated to 0 rel-err on all significant entries):

  a_val, b_val  = argmax of the 256-wide one-hots per byte
  xl = (a%16 + b%16), xh = (a>>4 + b>>4)           per byte, in [0,30]
  carry state c in {0, 0.5, 1}, init 0.5, over 8 nibbles (lo0,hi0,...,hi3):
      add = (c == 1); y = x + add; U = y mod 16; P = (c == 0.5)
      c' = clamp(x + c - 15, 0, 1)
  nibble dist = onehot(U)*(1-P/2) + onehot((U+1) mod 16)*(P/2)
  out byte row [256] = outer(h_dist, l_dist) flattened

Sharding: pure data parallel over the batch dim across 8 NeuronCores.
Per-core: 32 row-tiles of 128 in 2 chunks (extraction + carry chain per
chunk), nibble distributions in 4-tile sub-chunks, outer products fused
over tile pairs. Outers run on GPSIMD except the final sub-chunks, which
use the (by then idle) vector engine to shorten the tail.
"""

import numpy as np

import concourse.bass as bass
import concourse.bacc as bacc
import concourse.mybir as mybir
from concourse.tile import TileContext
from concourse.bass_utils import run_bass_kernel_spmd

N_CORES = 8
B_FULL = 32768
ROWS = B_FULL // N_CORES  # 4096 rows per core
F = 1024  # 4 bytes x 256 one-hot
P = 128
TILES_PER_CHUNK = 16
SUB = 4  # tiles per distribution sub-chunk
TAIL_VEC_SUBS = 2  # last-chunk sub-chunks whose outers run on DVE

FP = mybir.dt.float32
I32 = mybir.dt.int32


def _const_tables():
    k = np.arange(256)
    z = ((k % 16) + 32 * (k // 16)).astype(np.float32)
    # two bytes per dot: second byte's code scaled by 2^10 (sums stay exact
    # in f32: max 990*1024+990 < 2^24)
    ztab2 = np.concatenate([z, z * 1024.0])  # [512]
    ztab2 = np.broadcast_to(ztab2, (P, 512)).copy()
    # padded compare table: iota17b[j] = (j-1) mod 16. eq = [U == iota17b]
    # gives [U==k] at cols 1..16 and [U==15] at col 0, so cols 0..15 are
    # exactly [(U+1) mod 16 == k] -- one compare yields both one-hots.
    i17 = ((np.arange(17) + 15) % 16).astype(np.float32)
    iota17 = np.broadcast_to(i17, (P, 17)).copy()
    return ztab2, iota17


def build_nc(rows=ROWS):
    nt = rows // P
    ntc = min(TILES_PER_CHUNK, nt)
    assert nt % ntc == 0 and ntc % SUB == 0
    n_chunks = nt // ntc
    nsub = ntc // SUB

    # Bacc (not raw Bass): its compile pass legalizes multi-wait sync;
    # this walrus build allows only one embedded wait per instruction.
    nc = bacc.Bacc()
    # a and b are concatenated host-side so each tile needs a single DMA.
    ab_d = nc.declare_dram_parameter("ab", [2 * rows, F], FP, isOutput=False)
    ztab_d = nc.declare_dram_parameter("ztab2", [P, 512], FP, isOutput=False)
    iota_d = nc.declare_dram_parameter("iota17", [P, 17], FP, isOutput=False)
    out_d = nc.declare_dram_parameter("out", [rows, F], FP, isOutput=True)

    ab_v = ab_d[:, :].rearrange("(j t p) f -> t p j f", j=2, p=P)
    # paired output view: [pair u] -> [p, t2, f]
    out2_v = out_d[:, :].rearrange("(u t2 p) f -> u p t2 f", t2=2, p=P)

    AL = mybir.AluOpType

    with TileContext(nc) as tc:
        with (
            tc.tile_pool(name="consts", bufs=1) as cpool,
            tc.tile_pool(name="io", bufs=6) as iopool,
            tc.tile_pool(name="s", bufs=4) as spool,
            tc.tile_pool(name="scratch", bufs=4) as scpool,
            tc.tile_pool(name="arrs", bufs=2) as apool,
            tc.tile_pool(name="dist", bufs=3) as dpool,
            tc.tile_pool(name="outp", bufs=4) as opool,
        ):
            ztab_raw = cpool.tile([P, 512], FP, tag="ztab_raw")
            ztab = cpool.tile([P, 512], FP, tag="ztab")
            iota_raw = cpool.tile([P, 17], FP, tag="iota_raw")
            iota17 = cpool.tile([P, 17], FP, tag="iota17")
            nc.sync.dma_start(ztab_raw[:, :], ztab_d[:, :])
            nc.sync.dma_start(iota_raw[:, :], iota_d[:, :])
            # pre-touch consts on DVE so compute ops only wait on DVE state
            nc.vector.tensor_copy(ztab[:, :], ztab_raw[:, :])
            nc.vector.tensor_copy(iota17[:, :], iota_raw[:, :])

            # out-DMAs of chunk k are emitted after chunk k+1's input DMAs so
            # they never head-of-line block the input stream on the SP queue
            pending_outs = []
            for ch in range(n_chunks):
                t0 = ch * ntc
                z2 = apool.tile([P, 2 * ntc], FP, tag="z2")
                z2_i = apool.tile([P, 2 * ntc], I32, tag="z2i")
                zb_i = apool.tile([P, 4 * ntc], I32, tag="zbi")
                xlo_i = apool.tile([P, 4 * ntc], I32, tag="xloi")
                xhi_i = apool.tile([P, 4 * ntc], I32, tag="xhii")
                xnib = apool.tile([P, 8 * ntc], FP, tag="xnib")
                c_hist = apool.tile([P, 9 * ntc], FP, tag="chist")
                ctmp = apool.tile([P, ntc], FP, tag="ctmp")
                add_all = apool.tile([P, 8 * ntc], FP, tag="add")
                p_all = apool.tile([P, 8 * ntc], FP, tag="pall")
                y_all = apool.tile([P, 8 * ntc], FP, tag="yall")
                wrap = apool.tile([P, 8 * ntc], FP, tag="wrap")
                u_all = apool.tile([P, 8 * ntc], FP, tag="uall")
                w0_all = apool.tile([P, 8 * ntc], FP, tag="w0")
                w1_all = apool.tile([P, 8 * ntc], FP, tag="w1")

                # ---- phase 1: load + s=a+b + byte-pair dots -> z2 ----
                for t in range(ntc):
                    ab_t = iopool.tile([P, 2 * F], FP, tag="ab")
                    ab_tv = ab_t[:, :].rearrange("p (j f) -> p j f", j=2)
                    nc.sync.dma_start(ab_tv, ab_v[t0 + t])
                    s_t = spool.tile([P, F], FP, tag="s")
                    # s on DVE: offloading to gpsimd stalls the dependent dot
                    # ops (DVE stream is FIFO; embedded waits block it), which
                    # measured slower every time despite the freed cycles.
                    nc.vector.tensor_add(s_t[:, :], ab_t[:, 0:F], ab_t[:, F : 2 * F])
                    for i2 in range(2):
                        prod = scpool.tile([P, 512], FP, tag="prod")
                        # accum = dot(s bytes [2i2,2i2+1], ztab2)
                        nc.vector.scalar_tensor_tensor(
                            out=prod[:, :],
                            in0=s_t[:, i2 * 512 : (i2 + 1) * 512],
                            scalar=1.0,
                            in1=ztab[:, :],
                            op0=AL.mult,
                            op1=AL.mult,
                            accum_out=z2[:, i2 * ntc + t : i2 * ntc + t + 1],
                        )
                for u_idx, o2p in pending_outs:
                    nc.sync.dma_start(out2_v[u_idx], o2p[:, :])
                pending_outs = []

                # ---- phase 2: split z2 -> per-byte nibble sums xnib ----
                nc.vector.tensor_copy(z2_i[:, :], z2[:, :])  # f32 -> i32 exact
                zb_v = zb_i[:, :].rearrange("p (i2 par t) -> p i2 par t", par=2, t=ntc)
                z2_v = z2_i[:, :].rearrange("p (i2 t) -> p i2 t", t=ntc)
                nc.vector.tensor_scalar(
                    out=zb_v[:, :, 0, :], in0=z2_v, scalar1=1023, scalar2=None,
                    op0=AL.bitwise_and,
                )
                nc.vector.tensor_scalar(
                    out=zb_v[:, :, 1, :], in0=z2_v, scalar1=10, scalar2=None,
                    op0=AL.logical_shift_right,
                )
                nc.vector.tensor_scalar(
                    out=xlo_i[:, :], in0=zb_i[:, :], scalar1=31, scalar2=None,
                    op0=AL.bitwise_and,
                )
                nc.vector.tensor_scalar(
                    out=xhi_i[:, :], in0=zb_i[:, :], scalar1=5, scalar2=None,
                    op0=AL.logical_shift_right,
                )
                xnib_v = xnib[:, :].rearrange("p (i two t) -> p i two t", two=2, t=ntc)
                nc.vector.tensor_copy(
                    xnib_v[:, :, 0, :],
                    xlo_i[:, :].rearrange("p (i t) -> p i t", t=ntc),
                )
                nc.vector.tensor_copy(
                    xnib_v[:, :, 1, :],
                    xhi_i[:, :].rearrange("p (i t) -> p i t", t=ntc),
                )

                # ---- phase 3: sequential carry chain over 8 nibbles ----
                nc.vector.memset(c_hist[:, 0:ntc], 0.5)
                for n in range(8):
                    x_n = xnib[:, n * ntc : (n + 1) * ntc]
                    c_in = c_hist[:, n * ntc : (n + 1) * ntc]
                    c_out = c_hist[:, (n + 1) * ntc : (n + 2) * ntc]
                    nc.vector.scalar_tensor_tensor(
                        out=ctmp[:, :], in0=x_n, scalar=-15.0, in1=c_in,
                        op0=AL.add, op1=AL.add,
                    )
                    nc.vector.tensor_scalar(
                        out=c_out, in0=ctmp[:, :], scalar1=0.0, scalar2=1.0,
                        op0=AL.max, op1=AL.min,
                    )

                # ---- phase 4: vectorized U/P/weights over all nibbles ----
                c_pre = c_hist[:, 0 : 8 * ntc]
                nc.vector.tensor_scalar(
                    out=add_all[:, :], in0=c_pre, scalar1=0.75, scalar2=None,
                    op0=AL.is_ge,
                )
                nc.vector.tensor_scalar(
                    out=p_all[:, :], in0=c_pre, scalar1=0.5, scalar2=None,
                    op0=AL.is_equal,
                )
                nc.vector.tensor_add(y_all[:, :], xnib[:, :], add_all[:, :])
                nc.vector.tensor_scalar(
                    out=wrap[:, :], in0=y_all[:, :], scalar1=15.5, scalar2=None,
                    op0=AL.is_ge,
                )
                nc.vector.scalar_tensor_tensor(
                    out=u_all[:, :], in0=wrap[:, :], scalar=-16.0, in1=y_all[:, :],
                    op0=AL.mult, op1=AL.add,
                )
                nc.vector.tensor_scalar(
                    out=w1_all[:, :], in0=p_all[:, :], scalar1=0.5, scalar2=None,
                    op0=AL.mult,
                )
                nc.vector.tensor_scalar(
                    out=w0_all[:, :], in0=p_all[:, :], scalar1=-0.5, scalar2=1.0,
                    op0=AL.mult, op1=AL.add,
                )

                # ---- phases 5+6 per sub-chunk: dists then paired outers ----
                u_nv = u_all[:, :].rearrange("p (n t) -> p n t", t=ntc)
                w0_nv = w0_all[:, :].rearrange("p (n t) -> p n t", t=ntc)
                w1_nv = w1_all[:, :].rearrange("p (n t) -> p n t", t=ntc)
                for sb in range(nsub):
                    ts0 = sb * SUB
                    shape17 = [P, 8, SUB, 17]
                    shape16 = [P, 8, SUB, 16]
                    iota_b = iota17[:, None, None, :].broadcast_to(shape17)
                    u_b = u_nv[:, :, ts0 : ts0 + SUB][:, :, :, None].broadcast_to(shape17)
                    w0_b = w0_nv[:, :, ts0 : ts0 + SUB][:, :, :, None].broadcast_to(shape16)
                    w1_b = w1_nv[:, :, ts0 : ts0 + SUB][:, :, :, None].broadcast_to(shape16)
                    eqx = dpool.tile([P, 8 * SUB * 17], FP, tag="eqx")
                    dsub = dpool.tile([P, 8 * SUB * 16], FP, tag="dsub")
                    dtmp = dpool.tile([P, 8 * SUB * 16], FP, tag="dtmp")
                    eqx_v = eqx[:, :].rearrange("p (n t k) -> p n t k", t=SUB, k=17)
                    dsub_v = dsub[:, :].rearrange("p (n t k) -> p n t k", t=SUB, k=16)
                    dtmp_v = dtmp[:, :].rearrange("p (n t k) -> p n t k", t=SUB, k=16)
                    # dist build stays fully on DVE: moving the muls to
                    # gpsimd (cross-engine ping-pong) measured slower.
                    # eqx[.., j] = [U == (j-1) mod 16]:
                    #   cols 1..16 = onehot(U), cols 0..15 = onehot((U+1)%16)
                    nc.vector.tensor_tensor(eqx_v, u_b, iota_b, op=AL.is_equal)
                    nc.vector.tensor_mul(dsub_v, eqx_v[:, :, :, 1:17], w0_b)
                    nc.vector.tensor_mul(dtmp_v, eqx_v[:, :, :, 0:16], w1_b)
                    nc.vector.tensor_add(dsub[:, :], dsub[:, :], dtmp[:, :])

                    dv = dsub[:, :].rearrange(
                        "p (i par t k) -> p i par t k", par=2, t=SUB, k=16
                    )
                    last_subs = (ch == n_chunks - 1) and (sb >= nsub - TAIL_VEC_SUBS)
                    eng = nc.vector if last_subs else nc.gpsimd
                    for tp in range(SUB // 2):
                        tl = tp * 2
                        o2 = opool.tile([P, 2 * F], FP, tag="o2")
                        for t2 in range(2):  # TT allows max 3 free dims
                            o_v = o2[:, t2 * F : (t2 + 1) * F].rearrange(
                                "p (i h k) -> p i h k", h=16, k=16
                            )
                            h_b = dv[:, :, 1, tl + t2, :][:, :, :, None].broadcast_to(
                                [P, 4, 16, 16])
                            l_b = dv[:, :, 0, tl + t2, :][:, :, None, :].broadcast_to(
                                [P, 4, 16, 16])
                            eng.tensor_mul(o_v, h_b, l_b)
                        u_idx = (t0 + ts0 + tl) // 2
                        if ch == n_chunks - 1:
                            nc.sync.dma_start(out2_v[u_idx], o2[:, :])
                        else:
                            pending_outs.append((u_idx, o2))

    nc.finalize()
    return nc


_NC_CACHE = {}
LAST_RESULT = None


def kernel(**inputs) -> np.ndarray:
    global LAST_RESULT
    a = np.ascontiguousarray(np.asarray(inputs["a"], dtype=np.float32)).reshape(B_FULL, F)
    b = np.ascontiguousarray(np.asarray(inputs["b"], dtype=np.float32)).reshape(B_FULL, F)
    ztab2, iota17 = _const_tables()

    if ROWS not in _NC_CACHE:
        _NC_CACHE[ROWS] = build_nc(ROWS)
    nc = _NC_CACHE[ROWS]

    in_maps = []
    for c in range(N_CORES):
        ab = np.concatenate(
            [a[c * ROWS : (c + 1) * ROWS], b[c * ROWS : (c + 1) * ROWS]], axis=0
        )
        in_maps.append({
            "ab": np.ascontiguousarray(ab),
            "ztab2": ztab2,
            "iota17": iota17,
        })
    res = run_bass_kernel_spmd(nc, in_maps, core_ids=list(range(N_CORES)))
    LAST_RESULT = res
    out = np.concatenate([r["out"] for r in res.results], axis=0)
    return out.reshape(B_FULL, 4, 256)



# revision 2
# speedup vs baseline: 1.4178x; 1.4178x over previous
"""Trainium2 Bass kernel for nn_NeuralALU (batched byte-encoded 32-bit add).

The reference network is a chain of table-lookup matmuls + sharp softmaxes
(scale=100) over exactly-one-hot byte encodings. Because the inputs are exact
one-hots, the float pipeline collapses to a discrete algorithm (validated to
float-exactness against the jax reference in validate_math.py):

  z[2j]   = a_j%16 + b_j%16        (lo nibble sum of byte j, 0..30)
  z[2j+1] = a_j//16 + b_j//16      (hi nibble sum)
  carry c in {0, 0.5, 1}, init 0.5, chained over nibbles lo0,hi0,...,hi3:
      c' = max(g, min(p, c)),  g = [x>=16], p = [x>=15]
  add = [c==1]; y = x+add; U = y mod 16; Pf = [c==0.5]
  nibble dist = onehot(U)*(1-Pf/2) + onehot(U+1 mod 16)*(Pf/2)
  out byte row [256] = outer(hi_dist, lo_dist)

All staged values (0/1 one-hots, nibble sums <=31, dist weights {0,.5,1},
outputs {0,.25,.5,1}) are exactly representable in bf16, so inputs are staged
to the device as bf16 and outputs come back as bf16 — value-identical to the
fp32 computation, at half the HBM traffic.

Layout/engine strategy (pure data parallel over batch, 4096 rows/core):
 - Host pre-transposes each core's inputs to abT [2048 feat, 4096 rows] bf16
   so the (otherwise idle) tensor engine does the whole one-hot->nibble-sum
   extraction as 16 accumulating matmuls per 512-row block against an
   [feat, 8]-column nibble-value table (exact small-int arithmetic in fp32
   PSUM).
 - ScalarE (also otherwise idle) drains PSUM; TensorE transposes z back to
   rows-on-partitions; the carry chain is ONE tensor_tensor_scan; dist build
   on DVE; the 256-wide outer products are split DVE/GPSIMD; outputs stream
   back as bf16 on the ACT HWDGE ring (inputs use the SP ring).
"""

import numpy as np
import ml_dtypes

import concourse.bass as bass
import concourse.bacc as bacc
import concourse.mybir as mybir
from concourse.tile import TileContext
from concourse.bass_utils import run_bass_kernel_spmd

N_CORES = 8
B_FULL = 32768
ROWS = B_FULL // N_CORES  # 4096 rows per core
F = 1024                  # 4 bytes x 256 one-hot
P = 128
NTC = 8                   # row-tiles per chunk (1024 rows = one input group)
N_CHUNKS = ROWS // (P * NTC)  # 4
HG = 512                  # matmul moving free dim (rows per half-group)

FP = mybir.dt.float32
BF = mybir.dt.bfloat16

AL = None  # set in build


def _host_tables():
    # tab[f, m]: one-hot feature f (byte j=f//256, code k=f%256) contributes
    # k%16 to column 2j (lo) and k//16 to column 2j+1 (hi). Stored as 8
    # chunk-blocks of [128, 8] side by side -> [128, 64]; K-chunk c of the
    # concatenated [a;b] feature dim uses block c%8.
    f = np.arange(1024)
    j = f // 256
    k = f % 256
    tab = np.zeros((1024, 8), np.float32)
    tab[f, 2 * j] = (k % 16).astype(np.float32)
    tab[f, 2 * j + 1] = (k // 16).astype(np.float32)
    tab_blocks = tab.reshape(8, 128, 8).transpose(1, 0, 2).reshape(128, 64)
    ident = np.eye(8, dtype=np.float32)
    return (
        np.ascontiguousarray(tab_blocks).astype(ml_dtypes.bfloat16),
        np.ascontiguousarray(ident),
    )


def build_nc(rows=ROWS):
    global AL
    AL = mybir.AluOpType
    n_groups = rows // (P * NTC)

    nc = bacc.Bacc()
    abT_d = nc.declare_dram_parameter("abT", [2 * F, rows], BF, isOutput=False)
    tab_d = nc.declare_dram_parameter("tab", [P, 64], BF, isOutput=False)
    ident_d = nc.declare_dram_parameter("ident", [8, 8], FP, isOutput=False)
    out_d = nc.declare_dram_parameter("out", [rows, F], BF, isOutput=True)

    # input view: chunk c (of 16), partition p, row r  ->  abT[(c,p), r]
    ab_v = abT_d[:, :].rearrange("(c p) r -> p c r", p=P)
    # paired output view: pair u -> [p, t2, f]
    out2_v = out_d[:, :].rearrange("(u t2 p) f -> u p t2 f", t2=2, p=P)

    with TileContext(nc) as tc:
        with (
            tc.tile_pool(name="consts", bufs=1) as cpool,
            tc.tile_pool(name="io", bufs=2) as iopool,
            tc.tile_pool(name="zsb", bufs=3) as zpool,
            tc.tile_pool(name="arrs", bufs=2) as apool,
            tc.tile_pool(name="dist", bufs=2) as dpool,
            tc.tile_pool(name="outp", bufs=4) as opool,
            tc.tile_pool(name="psz", bufs=2, space="PSUM") as psumz,
            tc.tile_pool(name="pst", bufs=4, space="PSUM") as psumt,
        ):
            tab_sb = cpool.tile([P, 64], BF, tag="tab")
            ident_sb = cpool.tile([8, 8], FP, tag="ident")
            nc.sync.dma_start(tab_sb[:, :], tab_d[:, :])
            nc.sync.dma_start(ident_sb[:, :], ident_d[:, :])

            pending_tr = []  # (zsb_tile, xnib_tile, chunk-local half idx)

            def flush_transposes():
                for zsb, xnib, hh in pending_tr:
                    for tt in range(4):
                        tl = hh * 4 + tt  # chunk-local row-tile
                        pst = psumt.tile([P, 8], FP, tag="pst")
                        nc.tensor.transpose(
                            pst[:, :], zsb[:, tt * P : (tt + 1) * P], ident_sb[:, :]
                        )
                        nc.scalar.copy(xnib[:, tl * 8 : (tl + 1) * 8], pst[:, :])
                pending_tr.clear()

            for ch in range(n_groups):
                # ---- input group DMA (1024 rows x 2048 feats, 4 MiB) ----
                grp = iopool.tile([P, 16 * 2 * HG], BF, tag="grp")
                grp_v = grp[:, :].rearrange("p (c r) -> p c r", c=16)
                nc.sync.dma_start(
                    grp_v, ab_v[:, :, ch * 2 * HG : (ch + 1) * 2 * HG]
                )

                xnib = apool.tile([P, NTC * 8], BF, tag="xnib")

                # ---- extraction: 16 accumulating matmuls per 512 rows ----
                for hh in range(2):
                    psz = psumz.tile([8, HG], FP, tag="psz")
                    for c in range(16):
                        cc = c % 8
                        nc.tensor.matmul(
                            psz[:, :],
                            tab_sb[:, 8 * cc : 8 * cc + 8],
                            grp_v[:, c, hh * HG : (hh + 1) * HG],
                            start=(c == 0),
                            stop=(c == 15),
                        )
                    zsb = zpool.tile([8, HG], FP, tag="zsb")
                    nc.scalar.copy(zsb[:, :], psz[:, :])
                    pending_tr.append((zsb, xnib, hh))
                    if hh == 0:
                        continue
                    # defer this chunk's transposes until after next chunk's
                    # matmuls are queued? simpler: flush now — zcopy of hh=0
                    # long done, zcopy of hh=1 just issued; transposes wait
                    # only on that one.
                    flush_transposes()

                # ---- carry scan over [reset-padded] nibble chains ----
                # gp arrays [P, t, 9]: slots 0..7 = g/p of nibbles, slot 8 =
                # 0.5 reset so the scan restarts each row-tile... (slot 8 of
                # tile t feeds tile t+1's first nibble; chains are per-ROW,
                # tiles share partitions, so slots are per row-tile column
                # group along the free dim).
                gp_g = apool.tile([P, NTC * 9], BF, tag="gpg")
                gp_p = apool.tile([P, NTC * 9], BF, tag="gpp")
                cbuf = apool.tile([P, NTC * 9 + 1], BF, tag="cbuf")
                gg_v = gp_g[:, :].rearrange("p (t n) -> p t n", n=9)
                pp_v = gp_p[:, :].rearrange("p (t n) -> p t n", n=9)
                xn_v = xnib[:, :].rearrange("p (t n) -> p t n", n=8)
                nc.vector.tensor_scalar(
                    out=gg_v[:, :, 0:8], in0=xn_v, scalar1=15.5, scalar2=None,
                    op0=AL.is_ge,
                )
                nc.vector.tensor_scalar(
                    out=pp_v[:, :, 0:8], in0=xn_v, scalar1=14.5, scalar2=None,
                    op0=AL.is_ge,
                )
                nc.vector.memset(gg_v[:, :, 8:9], 0.5)
                nc.vector.memset(pp_v[:, :, 8:9], 0.5)
                nc.vector.memset(cbuf[:, 0:1], 0.5)
                nc.vector.tensor_tensor_scan(
                    out=cbuf[:, 1 : NTC * 9 + 1],
                    data0=gp_p[:, :],
                    data1=gp_g[:, :],
                    initial=0.5,
                    op0=AL.min,
                    op1=AL.max,
                )
                c_pre = cbuf[:, 0 : NTC * 9].rearrange("p (t n) -> p t n", n=9)[
                    :, :, 0:8
                ]

                # ---- U / P-flag / dist weights ----
                add_a = apool.tile([P, NTC * 8], BF, tag="adda")
                pf = apool.tile([P, NTC * 8], BF, tag="pf")
                y_a = apool.tile([P, NTC * 8], BF, tag="ya")
                wrap = apool.tile([P, NTC * 8], BF, tag="wrap")
                u_a = apool.tile([P, NTC * 8], BF, tag="ua")
                w0 = apool.tile([P, NTC * 8], BF, tag="w0")
                w1 = apool.tile([P, NTC * 8], BF, tag="w1")
                av = add_a[:, :].rearrange("p (t n) -> p t n", n=8)
                pv = pf[:, :].rearrange("p (t n) -> p t n", n=8)
                nc.vector.tensor_scalar(
                    out=av, in0=c_pre, scalar1=0.75, scalar2=None, op0=AL.is_ge
                )
                nc.vector.tensor_scalar(
                    out=pv, in0=c_pre, scalar1=0.5, scalar2=None, op0=AL.is_equal
                )
                nc.vector.tensor_add(y_a[:, :], xnib[:, :], add_a[:, :])
                nc.vector.tensor_scalar(
                    out=wrap[:, :], in0=y_a[:, :], scalar1=15.5, scalar2=None,
                    op0=AL.is_ge,
                )
                nc.vector.scalar_tensor_tensor(
                    out=u_a[:, :], in0=wrap[:, :], scalar=-16.0, in1=y_a[:, :],
                    op0=AL.mult, op1=AL.add,
                )
                nc.vector.tensor_scalar(
                    out=w1[:, :], in0=pf[:, :], scalar1=0.5, scalar2=None,
                    op0=AL.mult,
                )
                nc.vector.tensor_scalar(
                    out=w0[:, :], in0=pf[:, :], scalar1=-0.5, scalar2=1.0,
                    op0=AL.mult, op1=AL.add,
                )

                # ---- dist build: eqx via 17 per-k scalar compares (4x TS),
                #      then weighted combine into (t,n,k)-contiguous dsub ----
                TN = NTC * 8
                eqx = dpool.tile([P, 17 * TN], BF, tag="eqx")
                dsub = dpool.tile([P, TN * 16], BF, tag="dsub")
                dtmp = dpool.tile([P, TN * 16], BF, tag="dtmp")
                eqk_v = eqx[:, :].rearrange("p (k tn) -> p k tn", k=17)
                for kk in range(17):
                    nc.vector.tensor_scalar(
                        out=eqk_v[:, kk, :], in0=u_a[:, :],
                        scalar1=float((kk + 15) % 16), scalar2=None,
                        op0=AL.is_equal,
                    )
                eqt_v = eqx[:, :].rearrange("p (k tn) -> p tn k", k=17)
                ds_v = dsub[:, :].rearrange("p (tn k) -> p tn k", k=16)
                dt_v = dtmp[:, :].rearrange("p (tn k) -> p tn k", k=16)
                w0_b = w0[:, :, None].broadcast_to([P, TN, 16])
                w1_b = w1[:, :, None].broadcast_to([P, TN, 16])
                nc.vector.tensor_mul(ds_v, eqt_v[:, :, 1:17], w0_b)
                nc.vector.tensor_mul(dt_v, eqt_v[:, :, 0:16], w1_b)
                nc.vector.tensor_add(dsub[:, :], dsub[:, :], dtmp[:, :])

                # ---- outer products (paired tiles), split DVE/GPSIMD ----
                dv = dsub[:, :].rearrange(
                    "p (t i hf k) -> p t i hf k", i=4, hf=2, k=16
                )
                for tp in range(NTC // 2):
                    tl = tp * 2
                    u_idx = ch * (NTC // 2) + tp
                    # GPSIMD-heavy early, DVE picks up more near the tail
                    dve_pairs = (3,) if ch < 2 else (2, 3)
                    eng = nc.vector if tp in dve_pairs else nc.gpsimd
                    o2 = opool.tile([P, 2 * F], BF, tag="o2")
                    for t2 in range(2):
                        o_v = o2[:, t2 * F : (t2 + 1) * F].rearrange(
                            "p (i h k) -> p i h k", h=16, k=16
                        )
                        h_b = dv[:, tl + t2, :, 1, :][:, :, :, None].broadcast_to(
                            [P, 4, 16, 16]
                        )
                        l_b = dv[:, tl + t2, :, 0, :][:, :, None, :].broadcast_to(
                            [P, 4, 16, 16]
                        )
                        eng.tensor_mul(o_v, h_b, l_b)
                    # outputs ride the ACT HWDGE ring; inputs use SP's
                    nc.scalar.dma_start(out2_v[u_idx], o2[:, :])

    nc.finalize()
    return nc


_NC_CACHE = {}
LAST_RESULT = None


def kernel(**inputs) -> np.ndarray:
    global LAST_RESULT
    a = np.ascontiguousarray(np.asarray(inputs["a"], dtype=np.float32)).reshape(
        B_FULL, F
    )
    b = np.ascontiguousarray(np.asarray(inputs["b"], dtype=np.float32)).reshape(
        B_FULL, F
    )
    # bf16 staging: the one-hots are exactly 0.0/1.0, so the upper 16 bits of
    # each fp32 word ARE the bf16 encoding (lossless).
    a16 = a.view(np.uint16)[:, 1::2].reshape(N_CORES, ROWS, F)
    b16 = b.view(np.uint16)[:, 1::2].reshape(N_CORES, ROWS, F)
    aT = np.ascontiguousarray(a16.transpose(0, 2, 1))  # [8, 1024, 4096]
    bT = np.ascontiguousarray(b16.transpose(0, 2, 1))
    tab, ident = _host_tables()

    if ROWS not in _NC_CACHE:
        _NC_CACHE[ROWS] = build_nc(ROWS)
    nc = _NC_CACHE[ROWS]

    in_maps = []
    for c in range(N_CORES):
        abT = np.concatenate([aT[c], bT[c]], axis=0).view(ml_dtypes.bfloat16)
        in_maps.append({"abT": abT, "tab": tab, "ident": ident})
    res = run_bass_kernel_spmd(nc, in_maps, core_ids=list(range(N_CORES)))
    LAST_RESULT = res
    out16 = np.concatenate([r["out"] for r in res.results], axis=0)
    # bf16 -> fp32 exact expansion
    out32 = (out16.view(np.uint16).astype(np.uint32) << 16).view(np.float32)
    return out32.reshape(B_FULL, 4, 256)


# revision 8
# speedup vs baseline: 1.5384x; 1.0851x over previous
"""Trainium2 Bass kernel for nn_NeuralALU (batched byte-encoded 32-bit add).

The reference network is a chain of table-lookup matmuls + sharp softmaxes
(scale=100) over exactly-one-hot byte encodings. Because the inputs are exact
one-hots, the float pipeline collapses to a discrete algorithm (validated to
float-exactness against the jax reference in validate_math.py):

  z[2j]   = a_j%16 + b_j%16        (lo nibble sum of byte j, 0..30)
  z[2j+1] = a_j//16 + b_j//16      (hi nibble sum)
  carry c in {0, 0.5, 1}, init 0.5, chained over nibbles lo0,hi0,...,hi3:
      c' = max(g, min(p, c)),  g = [x>=16], p = [x>=15]
  add = [c==1]; y = x+add; U = y mod 16; Pf = [c==0.5]
  nibble dist = onehot(U)*(1-Pf/2) + onehot(U+1 mod 16)*(Pf/2)
  out byte row [256] = outer(hi_dist, lo_dist)

All staged values are exactly representable in the staging dtypes: the 0/1
one-hots and small-int nibble tables in fp8 e4m3, intermediate sums (<=31),
dist weights {0,.5,1} and outputs {0,.25,.5,1} in bf16 — so the kernel is
value-identical to the fp32 computation at a fraction of the HBM traffic.

Layout/engine strategy (pure data parallel over batch, 4096 rows/core):
 - Host pre-transposes each core's inputs to abT [2048 feat, 4096 rows] fp8
   so the (otherwise idle) tensor engine does the whole one-hot->nibble-sum
   extraction as fp8 DoubleRow matmuls (K=256 per instruction) against an
   [feat, 8]-column nibble-value table, accumulating exactly in fp32 PSUM.
 - ScalarE (also otherwise idle) drains PSUM; TensorE transposes z back to
   rows-on-partitions; the carry chain is ONE tensor_tensor_scan per chunk;
   dist build on DVE; the 256-wide outer products are split DVE/GPSIMD;
   outputs stream back as bf16 on the ACT HWDGE ring (inputs use SP's).
"""

import numpy as np
import ml_dtypes

import concourse.bass as bass
import concourse.bacc as bacc
import concourse.mybir as mybir
from concourse.tile import TileContext
from concourse.bass_utils import run_bass_kernel_spmd

N_CORES = 8
B_FULL = 32768
ROWS = B_FULL // N_CORES  # 4096 rows per core
F = 1024                  # 4 bytes x 256 one-hot
P = 128
NTC = 16                  # row-tiles per chunk (2048 rows = two input groups)
N_CHUNKS = ROWS // (P * NTC)  # 2
HG = 512                  # matmul moving free dim (rows per half-group)

FP = mybir.dt.float32
BF = mybir.dt.bfloat16
F8 = mybir.dt.float8e4

# outer-product pair assignment: which pairs (of 8 per chunk) run on DVE
DVE_PAIRS = {0: (1, 3, 5), 1: (1, 3, 5, 7)}


def _host_tables():
    # K-chunk c of abT covers features [128c, 128c+128) = half of one byte's
    # 256-wide one-hot block; its code k = 128*(c%2) + p. The per-chunk
    # [128, 2] nibble-value table (col 0: k%16, col 1: k//16) therefore only
    # depends on c's parity: tab[:, 2*(c%2) : 2*(c%2)+2].
    p = np.arange(128)
    tab = np.zeros((128, 4), np.float32)
    tab[:, 0] = p % 16
    tab[:, 1] = p // 16
    tab[:, 2] = p % 16
    tab[:, 3] = 8 + p // 16
    ident = np.eye(P, dtype=np.float32)
    return (
        np.ascontiguousarray(tab).astype(ml_dtypes.float8_e4m3),
        np.ascontiguousarray(ident),
    )


def build_nc(rows=ROWS):
    AL = mybir.AluOpType
    n_chunks = rows // (P * NTC)

    nc = bacc.Bacc()
    abT_d = nc.declare_dram_parameter("abT", [2 * F, rows], F8, isOutput=False)
    tab_d = nc.declare_dram_parameter("tab", [P, 4], F8, isOutput=False)
    ident_d = nc.declare_dram_parameter("ident", [P, P], FP, isOutput=False)
    out_d = nc.declare_dram_parameter("out", [rows, F], BF, isOutput=True)

    # input view: chunk c (of 16), partition p, row r  ->  abT[(c,p), r]
    ab_v = abT_d[:, :].rearrange("(c p) r -> p c r", p=P)
    # paired output view: pair u -> [p, t2, f]
    out2_v = out_d[:, :].rearrange("(u t2 p) f -> u p t2 f", t2=2, p=P)

    with TileContext(nc) as tc:
        with (
            tc.tile_pool(name="consts", bufs=1) as cpool,
            tc.tile_pool(name="io", bufs=2) as iopool,
            tc.tile_pool(name="zsb", bufs=3) as zpool,
            tc.tile_pool(name="arrs", bufs=2) as apool,
            tc.tile_pool(name="dist", bufs=2) as dpool,
            tc.tile_pool(name="outp", bufs=4) as opool,
            tc.tile_pool(name="psz", bufs=2, space="PSUM") as psumz,
            tc.tile_pool(name="pst", bufs=4, space="PSUM") as psumt,
        ):
            tab_sb = cpool.tile([P, 4], F8, tag="tab")
            ident_sb = cpool.tile([P, P], FP, tag="ident")
            nc.sync.dma_start(tab_sb[:, :], tab_d[:, :])
            nc.sync.dma_start(ident_sb[:, :], ident_d[:, :])

            pending_tr = []  # (zsb_tile, xnib_tile, chunk-local row-tile base)

            def flush_transposes():
                for zsb, xnib, tl0 in pending_tr:
                    for tt in range(4):
                        tl = tl0 + tt
                        pst = psumt.tile([P, P], FP, tag="pst")
                        nc.tensor.transpose(
                            pst[:, :], zsb[:, tt * P : (tt + 1) * P], ident_sb[:, :]
                        )
                        # z columns live at partitions {32j, 32j+1} per byte j
                        # -> strided view recovers carry order lo0,hi0,...
                        pz_v = pst[:, :].rearrange("p (g q) -> p g q", g=4)[
                            :, :, 0:2
                        ]
                        nc.scalar.copy(
                            xnib[:, tl * 8 : (tl + 1) * 8].rearrange(
                                "p (g q) -> p g q", g=4
                            ),
                            pz_v,
                        )
                pending_tr.clear()

            for ch in range(n_chunks):
                xnib = apool.tile([P, NTC * 8], BF, tag="xnib")

                for g in range(2):  # input groups of 1024 rows in this chunk
                    # ---- input group DMA (1024 rows x 2048 feats, 2 MiB),
                    #      chunk 0 split per half-group to shorten startup ----
                    grp = iopool.tile([P, 16 * 2 * HG], F8, tag="grp")
                    grp_v = grp[:, :].rearrange("p (c r) -> p c r", c=16)
                    r0 = (ch * 2 + g) * 2 * HG
                    if ch == 0 and g == 0:
                        for hh in range(2):
                            nc.sync.dma_start(
                                grp_v[:, :, hh * HG : (hh + 1) * HG],
                                ab_v[:, :, r0 + hh * HG : r0 + (hh + 1) * HG],
                            )
                    else:
                        nc.sync.dma_start(
                            grp_v, ab_v[:, :, r0 : r0 + 2 * HG]
                        )

                    # ---- extraction: 16 matmuls per 512 rows, packed 4-wide
                    #      into PE column groups {0,32,64,96} (byte j's four
                    #      K-chunks accumulate in its own column group) ----
                    for hh in range(2):
                        psz = psumz.tile([P, HG], FP, tag="psz")
                        for seq in range(4):
                            for j in range(4):
                                # chunks of byte j: a: 2j, 2j+1; b: 8+2j, 8+2j+1
                                c = (seq % 2) + 2 * j + 8 * (seq // 2)
                                nc.tensor.matmul(
                                    psz[32 * j : 32 * j + 2, :],
                                    tab_sb[:, 2 * (c % 2) : 2 * (c % 2) + 2],
                                    grp_v[:, c, hh * HG : (hh + 1) * HG],
                                    start=(seq == 0),
                                    stop=(seq == 3),
                                    skip_group_check=True,
                                    tile_position=(0, 32 * j),
                                )
                        zsb = zpool.tile([P, HG], FP, tag="zsb")
                        for j in range(4):
                            nc.scalar.copy(
                                zsb[32 * j : 32 * j + 2, :],
                                psz[32 * j : 32 * j + 2, :],
                            )
                        pending_tr.append((zsb, xnib, g * 8 + hh * 4))
                        if hh == 1:
                            flush_transposes()

                # ---- carry scan over [reset-padded] nibble chains ----
                gp_g = apool.tile([P, NTC * 9], BF, tag="gpg")
                gp_p = apool.tile([P, NTC * 9], BF, tag="gpp")
                cbuf = apool.tile([P, NTC * 9 + 1], BF, tag="cbuf")
                gg_v = gp_g[:, :].rearrange("p (t n) -> p t n", n=9)
                pp_v = gp_p[:, :].rearrange("p (t n) -> p t n", n=9)
                xn_v = xnib[:, :].rearrange("p (t n) -> p t n", n=8)
                nc.vector.tensor_scalar(
                    out=gg_v[:, :, 0:8], in0=xn_v, scalar1=15.5, scalar2=None,
                    op0=AL.is_ge,
                )
                nc.vector.tensor_scalar(
                    out=pp_v[:, :, 0:8], in0=xn_v, scalar1=14.5, scalar2=None,
                    op0=AL.is_ge,
                )
                nc.vector.memset(gg_v[:, :, 8:9], 0.5)
                nc.vector.memset(pp_v[:, :, 8:9], 0.5)
                nc.vector.memset(cbuf[:, 0:1], 0.5)
                nc.vector.tensor_tensor_scan(
                    out=cbuf[:, 1 : NTC * 9 + 1],
                    data0=gp_p[:, :],
                    data1=gp_g[:, :],
                    initial=0.5,
                    op0=AL.min,
                    op1=AL.max,
                )
                c_pre = cbuf[:, 0 : NTC * 9].rearrange("p (t n) -> p t n", n=9)[
                    :, :, 0:8
                ]

                # ---- U / P-flag / dist weights ----
                add_a = apool.tile([P, NTC * 8], BF, tag="adda")
                pf = apool.tile([P, NTC * 8], BF, tag="pf")
                y_a = apool.tile([P, NTC * 8], BF, tag="ya")
                wrap = apool.tile([P, NTC * 8], BF, tag="wrap")
                u_a = apool.tile([P, NTC * 8], BF, tag="ua")
                w0 = apool.tile([P, NTC * 8], BF, tag="w0")
                w1 = apool.tile([P, NTC * 8], BF, tag="w1")
                av = add_a[:, :].rearrange("p (t n) -> p t n", n=8)
                pv = pf[:, :].rearrange("p (t n) -> p t n", n=8)
                nc.vector.tensor_scalar(
                    out=av, in0=c_pre, scalar1=0.75, scalar2=None, op0=AL.is_ge
                )
                nc.vector.tensor_scalar(
                    out=pv, in0=c_pre, scalar1=0.5, scalar2=None, op0=AL.is_equal
                )
                nc.vector.tensor_add(y_a[:, :], xnib[:, :], add_a[:, :])
                nc.vector.tensor_scalar(
                    out=wrap[:, :], in0=y_a[:, :], scalar1=15.5, scalar2=None,
                    op0=AL.is_ge,
                )
                nc.vector.scalar_tensor_tensor(
                    out=u_a[:, :], in0=wrap[:, :], scalar=-16.0, in1=y_a[:, :],
                    op0=AL.mult, op1=AL.add,
                )
                nc.vector.tensor_scalar(
                    out=w1[:, :], in0=pf[:, :], scalar1=0.5, scalar2=None,
                    op0=AL.mult,
                )
                nc.vector.tensor_scalar(
                    out=w0[:, :], in0=pf[:, :], scalar1=-0.5, scalar2=1.0,
                    op0=AL.mult, op1=AL.add,
                )

                # ---- dist build: eqx via 17 per-k scalar compares (4x TS),
                #      then weighted combine into (t,n,k)-contiguous dsub ----
                TN = NTC * 8
                eqx = dpool.tile([P, 17 * TN], BF, tag="eqx")
                dsub = dpool.tile([P, TN * 16], BF, tag="dsub")
                dtmp = dpool.tile([P, TN * 16], BF, tag="dtmp")
                eqk_v = eqx[:, :].rearrange("p (k tn) -> p k tn", k=17)
                for kk in range(17):
                    nc.vector.tensor_scalar(
                        out=eqk_v[:, kk, :], in0=u_a[:, :],
                        scalar1=float((kk + 15) % 16), scalar2=None,
                        op0=AL.is_equal,
                    )
                eqt_v = eqx[:, :].rearrange("p (k tn) -> p tn k", k=17)
                ds_v = dsub[:, :].rearrange("p (tn k) -> p tn k", k=16)
                dt_v = dtmp[:, :].rearrange("p (tn k) -> p tn k", k=16)
                w0_b = w0[:, :, None].broadcast_to([P, TN, 16])
                w1_b = w1[:, :, None].broadcast_to([P, TN, 16])
                nc.vector.tensor_mul(ds_v, eqt_v[:, :, 1:17], w0_b)
                nc.vector.tensor_mul(dt_v, eqt_v[:, :, 0:16], w1_b)
                nc.vector.tensor_add(dsub[:, :], dsub[:, :], dtmp[:, :])

                # ---- outer products (paired tiles), split DVE/GPSIMD ----
                dv = dsub[:, :].rearrange(
                    "p (t i hf k) -> p t i hf k", i=4, hf=2, k=16
                )
                for tp in range(NTC // 2):
                    tl = tp * 2
                    u_idx = ch * (NTC // 2) + tp
                    eng = nc.vector if tp in DVE_PAIRS[ch] else nc.gpsimd
                    o2 = opool.tile([P, 2 * F], BF, tag="o2")
                    for t2 in range(2):
                        o_v = o2[:, t2 * F : (t2 + 1) * F].rearrange(
                            "p (i h k) -> p i h k", h=16, k=16
                        )
                        h_b = dv[:, tl + t2, :, 1, :][:, :, :, None].broadcast_to(
                            [P, 4, 16, 16]
                        )
                        l_b = dv[:, tl + t2, :, 0, :][:, :, None, :].broadcast_to(
                            [P, 4, 16, 16]
                        )
                        eng.tensor_mul(o_v, h_b, l_b)
                    # outputs ride the ACT HWDGE ring; inputs use SP's
                    nc.scalar.dma_start(out2_v[u_idx], o2[:, :])

    nc.finalize()
    return nc


_NC_CACHE = {}
LAST_RESULT = None


def kernel(**inputs) -> np.ndarray:
    global LAST_RESULT
    a = np.ascontiguousarray(np.asarray(inputs["a"], dtype=np.float32)).reshape(
        B_FULL, F
    )
    b = np.ascontiguousarray(np.asarray(inputs["b"], dtype=np.float32)).reshape(
        B_FULL, F
    )
    # fp8 e4m3 staging: the one-hots are exactly 0.0/1.0 -> bytes 0x00/0x38.
    a8 = (a.view(np.uint16)[:, 1::2] != 0).astype(np.uint8) * np.uint8(0x38)
    b8 = (b.view(np.uint16)[:, 1::2] != 0).astype(np.uint8) * np.uint8(0x38)
    aT = np.ascontiguousarray(a8.reshape(N_CORES, ROWS, F).transpose(0, 2, 1))
    bT = np.ascontiguousarray(b8.reshape(N_CORES, ROWS, F).transpose(0, 2, 1))
    tab, ident = _host_tables()

    if ROWS not in _NC_CACHE:
        _NC_CACHE[ROWS] = build_nc(ROWS)
    nc = _NC_CACHE[ROWS]

    in_maps = []
    for c in range(N_CORES):
        abT = np.concatenate([aT[c], bT[c]], axis=0).view(ml_dtypes.float8_e4m3)
        in_maps.append({"abT": abT, "tab": tab, "ident": ident})
    res = run_bass_kernel_spmd(nc, in_maps, core_ids=list(range(N_CORES)))
    LAST_RESULT = res
    out16 = np.concatenate([r["out"] for r in res.results], axis=0)
    # bf16 -> fp32 exact expansion
    out32 = (out16.view(np.uint16).astype(np.uint32) << 16).view(np.float32)
    return out32.reshape(B_FULL, 4, 256)


# revision 9
# speedup vs baseline: 1.6501x; 1.0726x over previous
"""Trainium2 Bass kernel for nn_NeuralALU (batched byte-encoded 32-bit add).

The reference network is a chain of table-lookup matmuls + sharp softmaxes
(scale=100) over exactly-one-hot byte encodings. Because the inputs are exact
one-hots, the float pipeline collapses to a discrete algorithm (validated to
float-exactness against the jax reference in validate_math.py):

  z[2j]   = a_j%16 + b_j%16        (lo nibble sum of byte j, 0..30)
  z[2j+1] = a_j//16 + b_j//16      (hi nibble sum)
  carry c in {0, 0.5, 1}, init 0.5, chained over nibbles lo0,hi0,...,hi3:
      c' = max(g, min(p, c)),  g = [x>=16], p = [x>=15]
  add = [c==1]; y = x+add; U = y mod 16; Pf = [c==0.5]
  nibble dist = onehot(U)*(1-Pf/2) + onehot(U+1 mod 16)*(Pf/2)
  out byte row [256] = outer(hi_dist, lo_dist)

All staged values are exactly representable in the staging dtypes: the 0/1
one-hots and small-int nibble tables in fp8 e4m3, intermediate sums (<=31),
dist weights {0,.5,1} and outputs {0,.25,.5,1} in bf16 — so the kernel is
value-identical to the fp32 computation at a fraction of the HBM traffic.

Layout/engine strategy (pure data parallel over batch, 4096 rows/core):
 - Host pre-transposes each core's inputs to abT [2048 feat, 4096 rows] fp8
   so the (otherwise idle) tensor engine does the whole one-hot->nibble-sum
   extraction as matmuls against tiny [128, 2] nibble-value tables, packed
   4-wide into PE column groups {0,32,64,96} via tile_position (byte j's four
   K-chunks accumulate in its own column group -> no cross-group reduction).
 - ScalarE (also otherwise idle) drains PSUM; TensorE transposes z back to
   rows-on-partitions; the carry chain is ONE tensor_tensor_scan per chunk;
   dist build on DVE; the 256-wide outer products are split DVE/GPSIMD in
   4-row-tile quads; outputs stream back as bf16 in 1 MiB DMAs on the ACT
   HWDGE ring (inputs use SP's).
"""

import numpy as np
import ml_dtypes

import concourse.bass as bass
import concourse.bacc as bacc
import concourse.mybir as mybir
from concourse.tile import TileContext
from concourse.bass_utils import run_bass_kernel_spmd

N_CORES = 8
B_FULL = 32768
ROWS = B_FULL // N_CORES  # 4096 rows per core
F = 1024                  # 4 bytes x 256 one-hot
P = 128
NTC = 8                   # row-tiles per chunk (1024 rows = one input group)
HG = 512                  # matmul moving free dim (rows per half-group)

FP = mybir.dt.float32
BF = mybir.dt.bfloat16
F8 = mybir.dt.float8e4

# which of each chunk's two 4-tile output quads run their outers on DVE
# (the rest go to GPSIMD); tuned from engine-occupancy profiles
DVE_QUADS = {0: (), 1: (1,), 2: (1,), 3: (1,)}


def _host_tables():
    # K-chunk c of abT covers features [128c, 128c+128) = half of one byte's
    # 256-wide one-hot block; its code k = 128*(c%2) + p. The per-chunk
    # [128, 2] nibble-value table (col 0: k%16, col 1: k//16) therefore only
    # depends on c's parity: tab[:, 2*(c%2) : 2*(c%2)+2].
    p = np.arange(128)
    tab = np.zeros((128, 4), np.float32)
    tab[:, 0] = p % 16
    tab[:, 1] = p // 16
    tab[:, 2] = p % 16
    tab[:, 3] = 8 + p // 16
    ident = np.eye(P, dtype=np.float32)
    # iota17[k] = (k+15) % 16 so one compare row yields onehot(U) at cols
    # 1..16 and onehot((U+1)%16) at cols 0..15
    i17 = ((np.arange(17) + 15) % 16).astype(np.float32)
    iota17 = np.broadcast_to(i17, (P, 17))
    return (
        np.ascontiguousarray(tab).astype(ml_dtypes.float8_e4m3),
        np.ascontiguousarray(ident),
        np.ascontiguousarray(iota17).astype(ml_dtypes.bfloat16),
    )


def build_nc(rows=ROWS):
    AL = mybir.AluOpType
    n_chunks = rows // (P * NTC)

    nc = bacc.Bacc()
    abT_d = nc.declare_dram_parameter("abT", [2 * F, rows], F8, isOutput=False)
    tab_d = nc.declare_dram_parameter("tab", [P, 4], F8, isOutput=False)
    ident_d = nc.declare_dram_parameter("ident", [P, P], FP, isOutput=False)
    iota_d = nc.declare_dram_parameter("iota17", [P, 17], BF, isOutput=False)
    out_d = nc.declare_dram_parameter("out", [rows, F], BF, isOutput=True)

    # input view: chunk c (of 16), partition p, row r  ->  abT[(c,p), r]
    ab_v = abT_d[:, :].rearrange("(c p) r -> p c r", p=P)
    # quad output view: quad u -> [p, t4, f] (4 row-tiles per DMA)
    out4_v = out_d[:, :].rearrange("(u t4 p) f -> u p t4 f", t4=4, p=P)

    with TileContext(nc) as tc:
        with (
            tc.tile_pool(name="consts", bufs=1) as cpool,
            tc.tile_pool(name="io", bufs=2) as iopool,
            tc.tile_pool(name="zsb", bufs=3) as zpool,
            tc.tile_pool(name="arrs", bufs=2) as apool,
            tc.tile_pool(name="dist", bufs=2) as dpool,
            tc.tile_pool(name="outp", bufs=3) as opool,
            tc.tile_pool(name="psz", bufs=2, space="PSUM") as psumz,
            tc.tile_pool(name="pst", bufs=4, space="PSUM") as psumt,
        ):
            tab_sb = cpool.tile([P, 4], F8, tag="tab")
            ident_sb = cpool.tile([P, P], FP, tag="ident")
            iota17 = cpool.tile([P, 17], BF, tag="iota17")
            nc.sync.dma_start(tab_sb[:, :], tab_d[:, :])
            nc.sync.dma_start(ident_sb[:, :], ident_d[:, :])
            nc.sync.dma_start(iota17[:, :], iota_d[:, :])

            pending_tr = []  # (zsb_tile, xnib_tile, chunk-local row-tile base)

            def flush_transposes():
                for zsb, xnib, tl0 in pending_tr:
                    for tt in range(4):
                        tl = tl0 + tt
                        pst = psumt.tile([P, P], FP, tag="pst")
                        nc.tensor.transpose(
                            pst[:, :], zsb[:, tt * P : (tt + 1) * P], ident_sb[:, :]
                        )
                        # z columns live at partitions {32j, 32j+1} per byte j
                        # -> strided view recovers carry order lo0,hi0,...
                        pz_v = pst[:, :].rearrange("p (g q) -> p g q", g=4)[
                            :, :, 0:2
                        ]
                        nc.scalar.copy(
                            xnib[:, tl * 8 : (tl + 1) * 8].rearrange(
                                "p (g q) -> p g q", g=4
                            ),
                            pz_v,
                        )
                pending_tr.clear()

            for ch in range(n_chunks):
                xnib = apool.tile([P, NTC * 8], BF, tag="xnib")

                # ---- input group DMA (1024 rows x 2048 feats, 2 MiB),
                #      chunk 0 split per half-group to shorten startup ----
                grp = iopool.tile([P, 16 * 2 * HG], F8, tag="grp")
                grp_v = grp[:, :].rearrange("p (c r) -> p c r", c=16)
                r0 = ch * 2 * HG
                if ch == 0:
                    for hh in range(2):
                        nc.sync.dma_start(
                            grp_v[:, :, hh * HG : (hh + 1) * HG],
                            ab_v[:, :, r0 + hh * HG : r0 + (hh + 1) * HG],
                        )
                else:
                    nc.sync.dma_start(grp_v, ab_v[:, :, r0 : r0 + 2 * HG])

                # ---- extraction: 16 matmuls per 512 rows, packed 4-wide
                #      into PE column groups {0,32,64,96} (byte j's four
                #      K-chunks accumulate in its own column group) ----
                for hh in range(2):
                    psz = psumz.tile([P, HG], FP, tag="psz")
                    for seq in range(4):
                        for j in range(4):
                            # chunks of byte j: a: 2j, 2j+1; b: 8+2j, 8+2j+1
                            c = (seq % 2) + 2 * j + 8 * (seq // 2)
                            nc.tensor.matmul(
                                psz[32 * j : 32 * j + 2, :],
                                tab_sb[:, 2 * (c % 2) : 2 * (c % 2) + 2],
                                grp_v[:, c, hh * HG : (hh + 1) * HG],
                                start=(seq == 0),
                                stop=(seq == 3),
                                skip_group_check=True,
                                tile_position=(0, 32 * j),
                            )
                    zsb = zpool.tile([P, HG], FP, tag="zsb")
                    nc.scalar.copy(zsb[:, :], psz[:, :])
                    pending_tr.append((zsb, xnib, hh * 4))
                    if hh == 1:
                        flush_transposes()

                # ---- carry scan over [reset-padded] nibble chains ----
                gp_g = apool.tile([P, NTC * 9], BF, tag="gpg")
                gp_p = apool.tile([P, NTC * 9], BF, tag="gpp")
                cbuf = apool.tile([P, NTC * 9 + 1], BF, tag="cbuf")
                gg_v = gp_g[:, :].rearrange("p (t n) -> p t n", n=9)
                pp_v = gp_p[:, :].rearrange("p (t n) -> p t n", n=9)
                xn_v = xnib[:, :].rearrange("p (t n) -> p t n", n=8)
                nc.vector.tensor_scalar(
                    out=gg_v[:, :, 0:8], in0=xn_v, scalar1=15.5, scalar2=None,
                    op0=AL.is_ge,
                )
                nc.vector.tensor_scalar(
                    out=pp_v[:, :, 0:8], in0=xn_v, scalar1=14.5, scalar2=None,
                    op0=AL.is_ge,
                )
                nc.vector.memset(gg_v[:, :, 8:9], 0.5)
                nc.vector.memset(pp_v[:, :, 8:9], 0.5)
                nc.vector.memset(cbuf[:, 0:1], 0.5)
                nc.vector.tensor_tensor_scan(
                    out=cbuf[:, 1 : NTC * 9 + 1],
                    data0=gp_p[:, :],
                    data1=gp_g[:, :],
                    initial=0.5,
                    op0=AL.min,
                    op1=AL.max,
                )
                c_pre = cbuf[:, 0 : NTC * 9].rearrange("p (t n) -> p t n", n=9)[
                    :, :, 0:8
                ]

                # ---- U / P-flag / dist weights ----
                add_a = apool.tile([P, NTC * 8], BF, tag="adda")
                pf = apool.tile([P, NTC * 8], BF, tag="pf")
                y_a = apool.tile([P, NTC * 8], BF, tag="ya")
                wrap = apool.tile([P, NTC * 8], BF, tag="wrap")
                u_a = apool.tile([P, NTC * 8], BF, tag="ua")
                w0 = apool.tile([P, NTC * 8], BF, tag="w0")
                w1 = apool.tile([P, NTC * 8], BF, tag="w1")
                av = add_a[:, :].rearrange("p (t n) -> p t n", n=8)
                pv = pf[:, :].rearrange("p (t n) -> p t n", n=8)
                nc.vector.tensor_scalar(
                    out=av, in0=c_pre, scalar1=0.75, scalar2=None, op0=AL.is_ge
                )
                nc.vector.tensor_scalar(
                    out=pv, in0=c_pre, scalar1=0.5, scalar2=None, op0=AL.is_equal
                )
                nc.vector.tensor_add(y_a[:, :], xnib[:, :], add_a[:, :])
                nc.vector.tensor_scalar(
                    out=wrap[:, :], in0=y_a[:, :], scalar1=15.5, scalar2=None,
                    op0=AL.is_ge,
                )
                nc.vector.scalar_tensor_tensor(
                    out=u_a[:, :], in0=wrap[:, :], scalar=-16.0, in1=y_a[:, :],
                    op0=AL.mult, op1=AL.add,
                )
                nc.vector.tensor_scalar(
                    out=w1[:, :], in0=pf[:, :], scalar1=0.5, scalar2=None,
                    op0=AL.mult,
                )
                nc.vector.tensor_scalar(
                    out=w0[:, :], in0=pf[:, :], scalar1=-0.5, scalar2=1.0,
                    op0=AL.mult, op1=AL.add,
                )

                # ---- dist build: one iota-compare then weighted combine ----
                TN = NTC * 8
                eqx = dpool.tile([P, TN * 17], BF, tag="eqx")
                dsub = dpool.tile([P, TN * 16], BF, tag="dsub")
                dtmp = dpool.tile([P, TN * 16], BF, tag="dtmp")
                # eqx[tn, k] = [U[tn] == (k+15)%16]: cols 1..16 = onehot(U),
                # cols 0..15 = onehot((U+1)%16)
                eq_v = eqx[:, :].rearrange("p (tn k) -> p tn k", k=17)
                u_b = u_a[:, :, None].broadcast_to([P, TN, 17])
                io_b = iota17[:, None, :].broadcast_to([P, TN, 17])
                nc.vector.tensor_tensor(eq_v, u_b, io_b, op=AL.is_equal)
                ds_v = dsub[:, :].rearrange("p (tn k) -> p tn k", k=16)
                dt_v = dtmp[:, :].rearrange("p (tn k) -> p tn k", k=16)
                w0_b = w0[:, :, None].broadcast_to([P, TN, 16])
                w1_b = w1[:, :, None].broadcast_to([P, TN, 16])
                nc.vector.tensor_mul(ds_v, eq_v[:, :, 1:17], w0_b)
                nc.vector.tensor_mul(dt_v, eq_v[:, :, 0:16], w1_b)
                nc.vector.tensor_add(dsub[:, :], dsub[:, :], dtmp[:, :])

                # ---- outer products in 4-row-tile quads, split DVE/GPSIMD ----
                dv = dsub[:, :].rearrange(
                    "p (t i hf k) -> p t i hf k", i=4, hf=2, k=16
                )
                for q in range(NTC // 4):
                    u_idx = ch * (NTC // 4) + q
                    eng = nc.vector if q in DVE_QUADS[ch] else nc.gpsimd
                    o4 = opool.tile([P, 4 * F], BF, tag="o4")
                    for t4 in range(4):
                        o_v = o4[:, t4 * F : (t4 + 1) * F].rearrange(
                            "p (i h k) -> p i h k", h=16, k=16
                        )
                        tl = q * 4 + t4
                        h_b = dv[:, tl, :, 1, :][:, :, :, None].broadcast_to(
                            [P, 4, 16, 16]
                        )
                        l_b = dv[:, tl, :, 0, :][:, :, None, :].broadcast_to(
                            [P, 4, 16, 16]
                        )
                        eng.tensor_mul(o_v, l_b, h_b)
                    # outputs ride the ACT HWDGE ring; inputs use SP's
                    nc.scalar.dma_start(out4_v[u_idx], o4[:, :])

    nc.finalize()
    return nc


_NC_CACHE = {}
LAST_RESULT = None


def kernel(**inputs) -> np.ndarray:
    global LAST_RESULT
    a = np.ascontiguousarray(np.asarray(inputs["a"], dtype=np.float32)).reshape(
        B_FULL, F
    )
    b = np.ascontiguousarray(np.asarray(inputs["b"], dtype=np.float32)).reshape(
        B_FULL, F
    )
    # fp8 e4m3 staging: the one-hots are exactly 0.0/1.0 -> bytes 0x00/0x38.
    a8 = (a.view(np.uint16)[:, 1::2] != 0).astype(np.uint8) * np.uint8(0x38)
    b8 = (b.view(np.uint16)[:, 1::2] != 0).astype(np.uint8) * np.uint8(0x38)
    aT = np.ascontiguousarray(a8.reshape(N_CORES, ROWS, F).transpose(0, 2, 1))
    bT = np.ascontiguousarray(b8.reshape(N_CORES, ROWS, F).transpose(0, 2, 1))
    tab, ident, iota17 = _host_tables()

    if ROWS not in _NC_CACHE:
        _NC_CACHE[ROWS] = build_nc(ROWS)
    nc = _NC_CACHE[ROWS]

    in_maps = []
    for c in range(N_CORES):
        abT = np.concatenate([aT[c], bT[c]], axis=0).view(ml_dtypes.float8_e4m3)
        in_maps.append({"abT": abT, "tab": tab, "ident": ident, "iota17": iota17})
    res = run_bass_kernel_spmd(nc, in_maps, core_ids=list(range(N_CORES)))
    LAST_RESULT = res
    out16 = np.concatenate([r["out"] for r in res.results], axis=0)
    # bf16 -> fp32 exact expansion
    out32 = (out16.view(np.uint16).astype(np.uint32) << 16).view(np.float32)
    return out32.reshape(B_FULL, 4, 256)
